# revision 63
# baseline (speedup 1.0000x reference)
"""Trainium2 Bass kernel for nn_AttSubLayerv2 (sparse_attention).

Math restructuring: scores = (Q K^T) @ R / sqrt(dk) is reassociated as
Q @ (K^T R) / sqrt(dk)  (contraction over dk=32 instead of S=2048), which
cuts matmul FLOPs ~32x and makes the problem memory-bound (dominated by the
268MB attn output).

Sharding: 16 (batch, head) pairs over 8 cores; core c handles b=c//4 and
heads {2*(c%4), 2*(c%4)+1}. The final fc contracts over all heads, so each
core produces a partial fc output which is ReduceScatter'd over the 4-core
group sharing a batch; each core then layernorms its 512-row slice.

Device dataflow is fully "transposed" (scoresT[n,q] layout):
  - KR[hd,n]   = K^T @ R: each core reads only its n-quarter of relation
                 (2MB bf16) and computes full-array (M=128) KR partials for
                 ALL 8 heads of its batch; one ReduceScatter over the 4-core
                 batch group exchanges head-slices (foreign quarter slots are
                 zeroed via a host-provided one-hot, keeping the program
                 SPMD-uniform)
  - sT[n,q]    = KR^T @ QT + maskneg (mask injected into PSUM via an
                 identity-weight matmul of a host-prepped fp8e5 -1024 mask;
                 exp underflows masked entries to exactly 0)
  - Em         = exp(sT / sqrt(dk))  (ACT reads 2 PSUM banks per instruction)
  - emv[33,q]  = [V | 1]^T @ Em      (softmax denominator fused via ones col)
  - attn[n,q]  = Em * (1/emv[32])    (written transposed; host un-transposes)
  - attT[dk,q] = emv[:32] * r; fc from attT natural layout.
Big matmuls run as float32r (FP22 multiply) for 1 cycle/row PE throughput.
Hot-loop DMAs are 1MB-batched to amortize the ~0.6us HWDGE descriptor
serialization; exp reads two PSUM banks per ACT instruction; normalize
multiplies are split DVE/GPSIMD.
"""
import sys
import numpy as np

sys.path.insert(0, "/opt/trn_rl_repo")

import ml_dtypes
import concourse.bass as bass
import concourse.bacc as bacc
import concourse.tile as tile
from concourse import mybir
from concourse import bass_utils
from concourse.bass_interp import get_hw_module
from concourse.masks import make_identity

B, S, D, H, DK = 2, 2048, 256, 8, 32
HPC = 2                # heads per core
HD = HPC * DK          # 64 head-dims per core
P = 128
NT = S // P            # 16 row tiles
QCW = 512              # q-chunk width
NQC = S // QCW         # 4
DC = D // P            # 2 d-chunks
LNR = S // 4           # 512 rows of layernorm output per core
GROUP = 4              # cores sharing a batch
SCALE = float(1.0 / np.sqrt(np.float32(DK)))
EPS = 1e-5

f32 = mybir.dt.float32
f32r = mybir.dt.float32r
bf16 = mybir.dt.bfloat16
f8e5 = mybir.dt.float8e5
FN = mybir.ActivationFunctionType
OP = mybir.AluOpType

_CACHED_NC = None


def _emit(tc, io, use_collective=True):
    nc = tc.nc
    q_b, k_b, v_b = io["q_b"], io["k_b"], io["v_b"]
    qsel = io["qsel"]
    rel_b, maskneg_t, q_rows = io["rel_b"], io["maskneg_t"], io["q_rows"]
    wq, wk, wv, wfc = io["wq"], io["wk"], io["wv"], io["wfc"]
    bfc, gamma, beta = io["bfc"], io["gamma"], io["beta"]
    attn_t, out_ln = io["attn_t"], io["out_ln"]

    with (
        nc.allow_low_precision(reason="fp32r (FP22) matmul inputs are deliberate"),
        tc.tile_pool(name="const", bufs=1) as const,
        tc.tile_pool(name="pers", bufs=1) as pers,
    ):
        ident_f32 = const.tile([P, P], f32)
        make_identity(nc, ident_f32)
        ident_f8 = const.tile([P, P], f8e5)
        nc.vector.tensor_copy(ident_f8, ident_f32)
        ones_f32 = const.tile([DK + 1, P], f32)
        nc.vector.memset(ones_f32, 1.0)
        ones_col = const.tile([DK + 1, P], f32r)
        nc.vector.tensor_copy(ones_col, ones_f32)
        onesv_f32 = const.tile([P, NT, HPC, 1], f32)
        nc.vector.memset(onesv_f32, 1.0)
        qsel_sb = const.tile([P, GROUP], f32)
        src_q = bass.AP(
            tensor=qsel.tensor, offset=qsel.offset, ap=[[0, P]] + qsel.ap
        )
        nc.gpsimd.dma_start(out=qsel_sb, in_=src_q)
        wq_sb = const.tile([P, DC, HD], f32r)
        wk_sb = const.tile([P, DC, D], f32r)
        wv_sb = const.tile([P, DC, HD], f32r)
        nc.sync.dma_start(wq_sb, wq.rearrange("(c p) h -> p c h", p=P))
        nc.sync.dma_start(wk_sb, wk.rearrange("(c p) h -> p c h", p=P))
        nc.sync.dma_start(wv_sb, wv.rearrange("(c p) h -> p c h", p=P))
        wfc_sb = const.tile([DK, HPC, D], f32r)
        nc.sync.dma_start(wfc_sb, wfc.rearrange("(j k) d -> k j d", j=HPC))

        K_sb = pers.tile([P, NT, D], bf16)
        V_sb = pers.tile([P, NT, HPC, DK + 1], f32r)
        QT_sb = pers.tile([HD, S], f32r)
        KR_sb = pers.tile([DK, HPC, S], f32r)
        attT_sb = pers.tile([DK, HPC, S], f32r)

        # ------- phase 0: projections (q/k/v arrive pre-transposed [D, S]) ---
        with (
            tc.tile_pool(name="xt", bufs=1) as xt_pool,
            tc.tile_pool(name="ph0ps", bufs=2, space="PSUM") as ph0ps,
        ):
            qT = xt_pool.tile([P, DC, S], f32r)
            kT = xt_pool.tile([P, DC, S], f32r)
            vT = xt_pool.tile([P, DC, S], f32r)
            nc.sync.dma_start(kT, k_b.rearrange("(c p) s -> p c s", p=P))
            nc.sync.dma_start(qT, q_b.rearrange("(c p) s -> p c s", p=P))
            nc.sync.dma_start(vT, v_b.rearrange("(c p) s -> p c s", p=P))
            # K[s, hd] first: the KR stream depends on it
            for t in range(NT):
                tsl = slice(t * P, (t + 1) * P)
                psk = ph0ps.tile([P, D], f32, tag="kps")
                for dc in range(DC):
                    nc.tensor.matmul(
                        psk,
                        lhsT=kT[:, dc, tsl],
                        rhs=wk_sb[:, dc, :],
                        start=(dc == 0),
                        stop=(dc == DC - 1),
                    )
                nc.vector.tensor_copy(K_sb[:, t, :], psk)
            # QT[hd, s] = Wq^T @ queryT  (both heads stacked on partitions)
            for qc in range(NQC):
                qsl = slice(qc * QCW, (qc + 1) * QCW)
                ps = ph0ps.tile([HD, QCW], f32, tag="qtps")
                for dc in range(DC):
                    nc.tensor.matmul(
                        ps,
                        lhsT=wq_sb[:, dc, :],
                        rhs=qT[:, dc, qsl],
                        start=(dc == 0),
                        stop=(dc == DC - 1),
                    )
                nc.vector.tensor_copy(QT_sb[:, qsl], ps)
            # V[s, hd]
            for t in range(NT):
                tsl = slice(t * P, (t + 1) * P)
                psv = ph0ps.tile([P, HD], f32, tag="vps")
                for dc in range(DC):
                    nc.tensor.matmul(
                        psv,
                        lhsT=vT[:, dc, tsl],
                        rhs=wv_sb[:, dc, :],
                        start=(dc == 0),
                        stop=(dc == DC - 1),
                    )
                for j in range(HPC):
                    nc.vector.tensor_copy(
                        V_sb[:, t, j, 0:DK], psv[:, j * DK:(j + 1) * DK]
                    )
            nc.vector.tensor_copy(V_sb[:, :, :, DK:DK + 1], onesv_f32)

        # ----- phases 1+2, explicitly interleaved at emission: KR streams per
        # ----- n-chunk, and the first q-chunk's two head-iterations are woven
        # ----- into the chunk loop so PE/ACT work hides the 16MB R stream ----
        KR_nc = [
            pers.tile([HD, QCW], f32r, name=f"KR_nc{c}") for c in range(NQC)
        ]
        SKEW = 3
        with (
            tc.tile_pool(name="mask", bufs=2) as mpool,
            tc.tile_pool(name="em", bufs=8) as empool,
            tc.tile_pool(name="smalls", bufs=3) as smalls,
            tc.tile_pool(name="fcout", bufs=2) as fcout,
            tc.tile_pool(name="dramfc", bufs=1, space="DRAM") as dpool_fc,
        ):
            fc_partial = dpool_fc.tile([S, D], f32)

            def load_mask(qc):
                qsl = slice(qc * QCW, (qc + 1) * QCW)
                mt = mpool.tile([P, NT, QCW], f8e5, tag="mt", name=f"mt{qc}")
                nc.sync.dma_start(
                    mt, maskneg_t[:, qsl].rearrange("(t p) q -> p t q", p=P)
                )
                return mt

            def em_at(em, t):
                return em[t // 4], t % 4

            def pair_head(qc, j, em, mt, t0):
                qsl = slice(qc * QCW, (qc + 1) * QCW)
                jsl = slice(j * DK, (j + 1) * DK)
                ps = scps.tile([P, 2, QCW], f32, tag="sc", name=f"sc{qc}_{j}_{t0}")
                for i, t in enumerate((t0, t0 + 1)):
                    nc.tensor.matmul(
                        ps[:, i, :],
                        lhsT=KR_nc[t // 4][jsl, (t % 4) * P:(t % 4 + 1) * P],
                        rhs=QT_sb[jsl, qsl],
                        start=True,
                        stop=False,
                    )
                    nc.tensor.matmul(
                        ps[:, i, :],
                        lhsT=ident_f8,
                        rhs=mt[:, t, :],
                        start=False,
                        stop=True,
                        skip_group_check=True,
                    )
                eh, tl = em_at(em, t0)
                nc.scalar.activation(eh[:, tl:tl + 2, :], ps, FN.Exp, scale=SCALE)

            def tile_attv(j, em, av_ps, t):
                eh, tl = em_at(em, t)
                nc.tensor.matmul(
                    av_ps,
                    lhsT=V_sb[:, t, j, :],
                    rhs=eh[:, tl, :],
                    start=(t == 0),
                    stop=(t == NT - 1),
                )

            def iter_tail(qc, j, em, av_ps):
                qsl = slice(qc * QCW, (qc + 1) * QCW)
                emv = smalls.tile([DK + 1, QCW], f32, tag="emv",
                                  name=f"emv{qc}_{j}")
                nc.vector.tensor_copy(emv, av_ps)
                rsb = smalls.tile([DK + 1, QCW], f32r, tag="rsb",
                                  name=f"rsb{qc}_{j}")
                nc.vector.reciprocal(rsb[DK:DK + 1, :], emv[DK:DK + 1, :])
                rb_ps = miscps.tile([P, QCW], f32, tag="misc", name=f"rb{qc}_{j}")
                nc.tensor.matmul(
                    rb_ps,
                    lhsT=ones_col[DK:DK + 1, :],
                    rhs=rsb[DK:DK + 1, :],
                    start=True,
                    stop=True,
                )
                rbc = smalls.tile([P, QCW], f32, tag="rbc", name=f"rbc{qc}_{j}")
                nc.vector.tensor_copy(rbc, rb_ps)
                nc.vector.tensor_mul(
                    attT_sb[:, j, qsl], emv[0:DK, :], rbc[0:DK, :]
                )
                for t in range(NT):
                    eh, tl = em_at(em, t)
                    if t % 2 == 1:
                        nc.gpsimd.tensor_mul(eh[:, tl, :], eh[:, tl, :], rbc)
                    else:
                        nc.vector.tensor_mul(eh[:, tl, :], eh[:, tl, :], rbc)
                    if t % 4 == 3:
                        tg = t // 4
                        nc.sync.dma_start(
                            attn_t[
                                j, tg * 4 * P:(tg + 1) * 4 * P, qsl
                            ].rearrange("(t p) q -> p t q", p=P),
                            em[tg].bitcast(f32),
                        )

            def emit_fc(qc):
                ot = fcout.tile([P, 4, D], f32, tag="fo", name=f"fo{qc}")
                for i, st in enumerate(range(qc * 4, (qc + 1) * 4)):
                    psf = miscps.tile([P, D], f32, tag="misc", name=f"fc{st}")
                    for j in range(HPC):
                        nc.tensor.matmul(
                            psf,
                            lhsT=attT_sb[:, j, st * P:(st + 1) * P],
                            rhs=wfc_sb[:, j, :],
                            start=(j == 0),
                            stop=(j == HPC - 1),
                        )
                    nc.vector.tensor_copy(ot[:, i, :], psf)
                nc.sync.dma_start(
                    fc_partial[qc * 4 * P:(qc + 1) * 4 * P, :].rearrange(
                        "(t p) d -> p t d", p=P
                    ),
                    ot,
                )

            # --- phase 1: KR partials on this core's n-quarter (full-array
            # --- M=128 matmuls over ALL 8 heads of its batch), exchanged via
            # --- one ReduceScatter; foreign quarter slots zeroed by the
            # --- host-provided one-hot qsel so the program stays SPMD ------
            from contextlib import ExitStack as _ES
            ph1 = _ES()
            rpool = ph1.enter_context(tc.tile_pool(name="rstream", bufs=2))
            krps = ph1.enter_context(
                tc.tile_pool(name="krps", bufs=1, space="PSUM")
            )
            krsb = ph1.enter_context(tc.tile_pool(name="krsb", bufs=1))
            rs_send = dpool_fc.tile([GROUP, HD, GROUP, QCW], f32)
            rs_recv = dpool_fc.tile([HD, GROUP, QCW], f32)
            kr_ps = [
                krps.tile([P, QCW], f32, tag=f"krp{h}", name=f"krp{h}")
                for h in range(2)
            ]
            for ktg in range(NT // 8):
                rt = rpool.tile([P, 8, QCW], bf16, tag="rt")
                nc.sync.dma_start(
                    rt,
                    rel_b[ktg * 8 * P:(ktg + 1) * 8 * P, :].rearrange(
                        "(t p) q -> p t q", p=P
                    ),
                )
                for i in range(8):
                    kt = ktg * 8 + i
                    for h in range(2):
                        nc.tensor.matmul(
                            kr_ps[h],
                            lhsT=K_sb[:, kt, h * P:(h + 1) * P],
                            rhs=rt[:, i, :],
                            start=(kt == 0),
                            stop=(kt == NT - 1),
                        )
            for h in range(2):
                ksb = krsb.tile([P, QCW], f32, tag="ks", name=f"ksb{h}")
                nc.vector.tensor_copy(ksb, kr_ps[h])
                st = krsb.tile([P, GROUP, QCW], f32, tag="st", name=f"st{h}")
                for g in range(GROUP):
                    nc.vector.tensor_scalar_mul(
                        st[:, g, :], ksb, qsel_sb[:, g:g + 1]
                    )
                nc.sync.dma_start(
                    rs_send[2 * h:2 * h + 2].rearrange("m h q w -> (m h) q w"),
                    st,
                )
            if use_collective:
                nc.gpsimd.collective_compute(
                    "ReduceScatter",
                    OP.add,
                    replica_groups=[[0, 1, 2, 3], [4, 5, 6, 7]],
                    ins=[rs_send.opt()],
                    outs=[rs_recv.opt()],
                )
            else:
                nc.sync.dma_start(rs_recv[:], rs_send[0])
            for s in range(NQC):
                nc.sync.dma_start(KR_nc[s], rs_recv[:, s, :].bitcast(f32r))
            ph1.close()

            # --- phase 2: uniform q-chunk pipeline ---
            ctx2 = _ES()
            scps = ctx2.enter_context(
                tc.tile_pool(name="scps", bufs=2, space="PSUM")
            )
            avps = ctx2.enter_context(
                tc.tile_pool(name="avps", bufs=2, space="PSUM")
            )
            miscps = ctx2.enter_context(
                tc.tile_pool(name="miscps", bufs=1, space="PSUM")
            )
            for qc in range(NQC):
                mt = load_mask(qc)
                for j in range(HPC):
                    em = [empool.tile([P, 4, QCW], f32r, tag="em",
                                      name=f"em{qc}_{j}_{h}") for h in range(4)]
                    av_ps = avps.tile([DK + 1, QCW], f32, tag="av",
                                      name=f"av{qc}_{j}")
                    for t in range(NT + SKEW):
                        if t < NT and t % 2 == 0:
                            pair_head(qc, j, em, mt, t)
                        if t >= SKEW:
                            tile_attv(j, em, av_ps, t - SKEW)
                    iter_tail(qc, j, em, av_ps)
                emit_fc(qc)
            ctx2.close()

        # ---------------- phase 3: ReduceScatter + layernorm ----------------
        with (
            tc.tile_pool(name="lnp", bufs=2) as lnp,
            tc.tile_pool(name="lnc", bufs=1) as lnc,
            tc.tile_pool(name="dram", bufs=1, space="DRAM") as dpool,
        ):
            rs_out = dpool.tile([LNR, D], f32)
            if use_collective:
                nc.gpsimd.collective_compute(
                    "ReduceScatter",
                    OP.add,
                    replica_groups=[[0, 1, 2, 3], [4, 5, 6, 7]],
                    ins=[fc_partial.opt()],
                    outs=[rs_out.opt()],
                )
            else:
                # timing-sim variant: stand in for the ReduceScatter with a
                # same-size DRAM->DRAM copy of this core's slice
                nc.sync.dma_start(rs_out[:], fc_partial[0:LNR, :])
            # layernorm over this core's 512-row slice
            rs_sb = lnc.tile([P, 4, D], f32)
            qr_sb = lnc.tile([P, 4, D], f32)
            nc.sync.dma_start(rs_sb, rs_out[:].rearrange("(t p) d -> p t d", p=P))
            nc.sync.dma_start(qr_sb, q_rows.rearrange("(t p) d -> p t d", p=P))
            bfc_bc = lnc.tile([P, D], f32)
            gamma_bc = lnc.tile([P, D], f32)
            beta_bc = lnc.tile([P, D], f32)
            for dst, vec in ((bfc_bc, bfc), (gamma_bc, gamma), (beta_bc, beta)):
                src = bass.AP(
                    tensor=vec.tensor, offset=vec.offset, ap=[[0, P]] + vec.ap
                )
                nc.gpsimd.dma_start(out=dst, in_=src)
            eps_t = lnc.tile([P, 1], f32)
            nc.vector.memset(eps_t, EPS)
            for t in range(4):
                x = lnp.tile([P, D], f32, tag="x")
                nc.vector.tensor_add(x, rs_sb[:, t, :], qr_sb[:, t, :])
                nc.vector.tensor_add(x, x, bfc_bc)
                stats = lnp.tile([P, 6], f32, tag="st")
                nc.vector.bn_stats(stats, x)
                mv = lnp.tile([P, 2], f32, tag="mv")
                nc.vector.bn_aggr(mv, stats)
                std = lnp.tile([P, 1], f32, tag="sd")
                nc.scalar.activation(std, mv[:, 1:2], FN.Sqrt, bias=eps_t)
                rstd = lnp.tile([P, 1], f32, tag="rs")
                nc.vector.reciprocal(rstd, std)
                xn = lnp.tile([P, D], f32, tag="xn")
                nc.vector.tensor_scalar(
                    xn, x, scalar1=mv[:, 0:1], scalar2=rstd,
                    op0=OP.subtract, op1=OP.mult,
                )
                nc.vector.tensor_mul(xn, xn, gamma_bc)
                nc.vector.tensor_add(xn, xn, beta_bc)
                nc.sync.dma_start(out_ln[t * P:(t + 1) * P, :], xn)


def _build(use_collective=True, n_iters=1):
    global _CACHED_NC
    if use_collective and n_iters == 1 and _CACHED_NC is not None:
        return _CACHED_NC
    nc = bacc.Bacc(
        "TRN2",
        target_bir_lowering=False,
        debug=False,
        enable_asserts=True,
        num_devices=8,
    )
    io = {}
    io["q_b"] = nc.dram_tensor("q_b", [D, S], f32r, kind="ExternalInput").ap()
    io["k_b"] = nc.dram_tensor("k_b", [D, S], f32r, kind="ExternalInput").ap()
    io["v_b"] = nc.dram_tensor("v_b", [D, S], f32r, kind="ExternalInput").ap()
    io["rel_b"] = nc.dram_tensor("rel_b", [S, QCW], bf16, kind="ExternalInput").ap()
    io["maskneg_t"] = nc.dram_tensor(
        "maskneg_t", [S, S], f8e5, kind="ExternalInput"
    ).ap()
    io["q_rows"] = nc.dram_tensor("q_rows", [LNR, D], f32, kind="ExternalInput").ap()
    io["wq"] = nc.dram_tensor("wq", [D, HD], f32r, kind="ExternalInput").ap()
    io["wk"] = nc.dram_tensor("wk", [D, D], f32r, kind="ExternalInput").ap()
    io["wv"] = nc.dram_tensor("wv", [D, HD], f32r, kind="ExternalInput").ap()
    io["wfc"] = nc.dram_tensor("wfc", [HD, D], f32r, kind="ExternalInput").ap()
    io["bfc"] = nc.dram_tensor("bfc", [D], f32, kind="ExternalInput").ap()
    io["qsel"] = nc.dram_tensor("qsel", [4], f32, kind="ExternalInput").ap()
    io["gamma"] = nc.dram_tensor("gamma", [D], f32, kind="ExternalInput").ap()
    io["beta"] = nc.dram_tensor("beta", [D], f32, kind="ExternalInput").ap()
    io["attn_t"] = nc.dram_tensor(
        "attn_t", [HPC, S, S], f32, kind="ExternalOutput"
    ).ap()
    io["out_ln"] = nc.dram_tensor("out_ln", [LNR, D], f32, kind="ExternalOutput").ap()

    with tile.TileContext(nc) as tc:
        for _ in range(n_iters):
            _emit(tc, io, use_collective=use_collective)
    nc.compile()
    nc.m = get_hw_module(nc.m)
    if use_collective and n_iters == 1:
        _CACHED_NC = nc
    return nc


def kernel(query, key_, value, mask, relation, Wq, Wk, Wv, Wfc, bfc, gamma, beta,
           _trace=False):
    query = np.asarray(query, np.float32)
    key_ = np.asarray(key_, np.float32)
    value = np.asarray(value, np.float32)
    mask = np.asarray(mask)
    relation = np.asarray(relation, np.float32)
    Wq = np.asarray(Wq, np.float32)
    Wk = np.asarray(Wk, np.float32)
    Wv = np.asarray(Wv, np.float32)
    Wfc = np.asarray(Wfc, np.float32)
    bfc = np.asarray(bfc, np.float32)
    gamma = np.asarray(gamma, np.float32)
    beta = np.asarray(beta, np.float32)

    nc = _build()

    maskneg = {}
    qkvT = {}
    relb16 = {}
    for b in range(B):
        maskneg[b] = np.ascontiguousarray(
            mask[b].T.astype(np.float32) * np.float32(-1024.0)
        ).astype(ml_dtypes.float8_e5m2)
        relb16[b] = np.ascontiguousarray(relation[b]).astype(ml_dtypes.bfloat16)
        qkvT[b] = (
            np.ascontiguousarray(query[b].T),
            np.ascontiguousarray(key_[b].T),
            np.ascontiguousarray(value[b].T),
        )

    in_maps = []
    for c in range(8):
        b, hp = c // 4, c % 4
        in_maps.append({
            "q_b": qkvT[b][0],
            "k_b": qkvT[b][1],
            "v_b": qkvT[b][2],
            "rel_b": np.ascontiguousarray(relb16[b][:, QCW * hp:QCW * (hp + 1)]),
            "maskneg_t": maskneg[b],
            "q_rows": np.ascontiguousarray(query[b, LNR * hp:LNR * (hp + 1)]),
            "wq": np.ascontiguousarray(Wq[:, HD * hp:HD * (hp + 1)]),
            "wk": Wk,
            "wv": np.ascontiguousarray(Wv[:, HD * hp:HD * (hp + 1)]),
            "wfc": np.ascontiguousarray(Wfc[HD * hp:HD * (hp + 1), :]),
            "bfc": bfc,
            "qsel": np.eye(4, dtype=np.float32)[hp],
            "gamma": gamma,
            "beta": beta,
        })

    res = bass_utils.run_bass_kernel_spmd(
        nc, in_maps, core_ids=list(range(8)), trace=_trace
    )

    attn_full = np.empty((B, H, S, S), np.float32)
    out_full = np.empty((B, S, D), np.float32)
    for c in range(8):
        b, hp = c // 4, c % 4
        r = res.results[c]
        for j in range(HPC):
            attn_full[b, HPC * hp + j] = r["attn_t"][j].T
        out_full[b, LNR * hp:LNR * (hp + 1)] = r["out_ln"]
    kernel._last_results = res
    return out_full, attn_full


# revision 67
# speedup vs baseline: 1.1000x; 1.1000x over previous
"""Trainium2 Bass kernel for nn_AttSubLayerv2 (sparse_attention).

Math restructuring: scores = (Q K^T) @ R / sqrt(dk) is reassociated as
Q @ (K^T R) / sqrt(dk)  (contraction over dk=32 instead of S=2048), which
cuts matmul FLOPs ~32x and makes the problem memory-bound (dominated by the
268MB attn output).

Sharding: 16 (batch, head) pairs over 8 cores; core c handles b=c//4 and
heads {2*(c%4), 2*(c%4)+1}. The final fc contracts over all heads, so each
core produces a partial fc output which is ReduceScatter'd over the 4-core
group sharing a batch; each core then layernorms its 512-row slice.

Device dataflow is fully "transposed" (scoresT[n,q] layout):
  - KR[hd,n]   = K^T @ R: each core reads only its n-quarter of relation
                 (2MB bf16) and computes full-array (M=128) KR partials for
                 ALL 8 heads of its batch; one ReduceScatter over the 4-core
                 batch group exchanges head-slices (foreign quarter slots are
                 zeroed via a host-provided one-hot, keeping the program
                 SPMD-uniform)
  - sT[n,q]    = KR^T @ QT + maskneg (mask injected into PSUM via an
                 identity-weight matmul of a host-prepped fp8e5 -1024 mask;
                 exp underflows masked entries to exactly 0)
  - Em         = exp(sT / sqrt(dk))  (ACT reads 2 PSUM banks per instruction)
  - emv[33,q]  = [V | 1]^T @ Em      (softmax denominator fused via ones col)
  - attn[n,q]  = Em * (1/emv[32])    (written transposed; host un-transposes)
  - attT[dk,q] = emv[:32] * r; fc from attT natural layout.
Big matmuls run as float32r (FP22 multiply) for 1 cycle/row PE throughput.
Hot-loop DMAs are 1MB-batched to amortize the ~0.6us HWDGE descriptor
serialization; exp reads two PSUM banks per ACT instruction; normalize
multiplies are split DVE/GPSIMD.
"""
import sys
import numpy as np

sys.path.insert(0, "/opt/trn_rl_repo")

import ml_dtypes
import concourse.bass as bass
import concourse.bacc as bacc
import concourse.tile as tile
from concourse import mybir
from concourse import bass_utils
from concourse.bass_interp import get_hw_module
from concourse.masks import make_identity

B, S, D, H, DK = 2, 2048, 256, 8, 32
HPC = 2                # heads per core
HD = HPC * DK          # 64 head-dims per core
P = 128
NT = S // P            # 16 row tiles
QCW = 512              # q-chunk width
NQC = S // QCW         # 4
DC = D // P            # 2 d-chunks
LNR = S // 4           # 512 rows of layernorm output per core
GROUP = 4              # cores sharing a batch
SCALE = float(1.0 / np.sqrt(np.float32(DK)))
EPS = 1e-5

f32 = mybir.dt.float32
f32r = mybir.dt.float32r
bf16 = mybir.dt.bfloat16
f8e5 = mybir.dt.float8e5
FN = mybir.ActivationFunctionType
OP = mybir.AluOpType

_CACHED_NC = None


def _emit(tc, io, use_collective=True):
    nc = tc.nc
    q_b, k_b, v_b = io["q_b"], io["k_b"], io["v_b"]
    qsel = io["qsel"]
    rel_b, maskneg_t, q_rows = io["rel_b"], io["maskneg_t"], io["q_rows"]
    wq, wk, wv, wfc = io["wq"], io["wk"], io["wv"], io["wfc"]
    bfc, gamma, beta = io["bfc"], io["gamma"], io["beta"]
    attn_t, out_ln = io["attn_t"], io["out_ln"]

    with (
        nc.allow_low_precision(reason="fp32r (FP22) matmul inputs are deliberate"),
        tc.tile_pool(name="const", bufs=1) as const,
        tc.tile_pool(name="pers", bufs=1) as pers,
    ):
        ident_f32 = const.tile([P, P], f32)
        make_identity(nc, ident_f32)
        ident_f8 = const.tile([P, P], f8e5)
        nc.vector.tensor_copy(ident_f8, ident_f32)
        ones_f32 = const.tile([DK + 1, P], f32)
        nc.vector.memset(ones_f32, 1.0)
        ones_col = const.tile([DK + 1, P], f32r)
        nc.vector.tensor_copy(ones_col, ones_f32)
        onesv_f32 = const.tile([P, NT, HPC, 1], f32)
        nc.vector.memset(onesv_f32, 1.0)
        qsel_sb = const.tile([P, GROUP], f32)
        src_q = bass.AP(
            tensor=qsel.tensor, offset=qsel.offset, ap=[[0, P]] + qsel.ap
        )
        nc.gpsimd.dma_start(out=qsel_sb, in_=src_q)
        wq_sb = const.tile([P, DC, HD], f32r)
        wk_sb = const.tile([P, DC, D], f32r)
        wv_sb = const.tile([P, DC, HD], f32r)
        nc.sync.dma_start(wq_sb, wq.rearrange("(c p) h -> p c h", p=P))
        nc.sync.dma_start(wk_sb, wk.rearrange("(c p) h -> p c h", p=P))
        nc.sync.dma_start(wv_sb, wv.rearrange("(c p) h -> p c h", p=P))
        wfc_sb = const.tile([DK, HPC, D], f32r)
        nc.sync.dma_start(wfc_sb, wfc.rearrange("(j k) d -> k j d", j=HPC))

        K_sb = pers.tile([P, NT, D], bf16)
        V_sb = pers.tile([P, NT, HPC, DK + 1], f32r)
        QT_sb = pers.tile([HD, S], f32r)
        KR_sb = pers.tile([DK, HPC, S], f32r)
        attT_sb = pers.tile([DK, HPC, S], f32r)

        # ------- phase 0: projections (q/k/v arrive pre-transposed [D, S]) ---
        with (
            tc.tile_pool(name="xt", bufs=1) as xt_pool,
            tc.tile_pool(name="ph0ps", bufs=2, space="PSUM") as ph0ps,
        ):
            qT = xt_pool.tile([P, DC, S], f32r)
            kT = xt_pool.tile([P, DC, S], f32r)
            vT = xt_pool.tile([P, DC, S], f32r)
            nc.sync.dma_start(kT, k_b.rearrange("(c p) s -> p c s", p=P))
            nc.sync.dma_start(qT, q_b.rearrange("(c p) s -> p c s", p=P))
            nc.sync.dma_start(vT, v_b.rearrange("(c p) s -> p c s", p=P))
            # K[s, hd] first: the KR stream depends on it
            for t in range(NT):
                tsl = slice(t * P, (t + 1) * P)
                psk = ph0ps.tile([P, D], f32, tag="kps")
                for dc in range(DC):
                    nc.tensor.matmul(
                        psk,
                        lhsT=kT[:, dc, tsl],
                        rhs=wk_sb[:, dc, :],
                        start=(dc == 0),
                        stop=(dc == DC - 1),
                    )
                nc.vector.tensor_copy(K_sb[:, t, :], psk)
            # QT[hd, s] = Wq^T @ queryT  (both heads stacked on partitions)
            for qc in range(NQC):
                qsl = slice(qc * QCW, (qc + 1) * QCW)
                ps = ph0ps.tile([HD, QCW], f32, tag="qtps")
                for dc in range(DC):
                    nc.tensor.matmul(
                        ps,
                        lhsT=wq_sb[:, dc, :],
                        rhs=qT[:, dc, qsl],
                        start=(dc == 0),
                        stop=(dc == DC - 1),
                    )
                nc.vector.tensor_copy(QT_sb[:, qsl], ps)
            # V[s, hd]
            for t in range(NT):
                tsl = slice(t * P, (t + 1) * P)
                psv = ph0ps.tile([P, HD], f32, tag="vps")
                for dc in range(DC):
                    nc.tensor.matmul(
                        psv,
                        lhsT=vT[:, dc, tsl],
                        rhs=wv_sb[:, dc, :],
                        start=(dc == 0),
                        stop=(dc == DC - 1),
                    )
                for j in range(HPC):
                    nc.vector.tensor_copy(
                        V_sb[:, t, j, 0:DK], psv[:, j * DK:(j + 1) * DK]
                    )
            nc.vector.tensor_copy(V_sb[:, :, :, DK:DK + 1], onesv_f32)

        # ----- phases 1+2, explicitly interleaved at emission: KR streams per
        # ----- n-chunk, and the first q-chunk's two head-iterations are woven
        # ----- into the chunk loop so PE/ACT work hides the 16MB R stream ----
        KR_nc = [
            pers.tile([HD, QCW], f32r, name=f"KR_nc{c}") for c in range(NQC)
        ]
        SKEW = 3
        with (
            tc.tile_pool(name="mask", bufs=2) as mpool,
            tc.tile_pool(name="em", bufs=8) as empool,
            tc.tile_pool(name="smalls", bufs=3) as smalls,
            tc.tile_pool(name="fcout", bufs=2) as fcout,
            tc.tile_pool(name="dramfc", bufs=1, space="DRAM") as dpool_fc,
        ):
            fc_partial = dpool_fc.tile([S, D], f32)

            def load_mask(qc):
                qsl = slice(qc * QCW, (qc + 1) * QCW)
                mt = mpool.tile([P, NT, QCW], f8e5, tag="mt", name=f"mt{qc}")
                # scalar-engine HWDGE ring: keeps the mask prefetch out of the
                # sync ring's in-order queue behind the attn out-stream
                nc.scalar.dma_start(
                    mt, maskneg_t[:, qsl].rearrange("(t p) q -> p t q", p=P)
                )
                return mt

            def em_at(em, t):
                return em[t // 4], t % 4

            def pair_head(qc, j, em, mt, t0):
                qsl = slice(qc * QCW, (qc + 1) * QCW)
                jsl = slice(j * DK, (j + 1) * DK)
                ps = scps.tile([P, 2, QCW], f32, tag="sc", name=f"sc{qc}_{j}_{t0}")
                for i, t in enumerate((t0, t0 + 1)):
                    nc.tensor.matmul(
                        ps[:, i, :],
                        lhsT=KR_nc[t // 4][jsl, (t % 4) * P:(t % 4 + 1) * P],
                        rhs=QT_sb[jsl, qsl],
                        start=True,
                        stop=False,
                    )
                    nc.tensor.matmul(
                        ps[:, i, :],
                        lhsT=ident_f8,
                        rhs=mt[:, t, :],
                        start=False,
                        stop=True,
                        skip_group_check=True,
                    )
                eh, tl = em_at(em, t0)
                nc.scalar.activation(eh[:, tl:tl + 2, :], ps, FN.Exp, scale=SCALE)

            def tile_attv(j, em, av_ps, t):
                eh, tl = em_at(em, t)
                nc.tensor.matmul(
                    av_ps,
                    lhsT=V_sb[:, t, j, :],
                    rhs=eh[:, tl, :],
                    start=(t == 0),
                    stop=(t == NT - 1),
                )

            def iter_tail(qc, j, em, av_ps):
                qsl = slice(qc * QCW, (qc + 1) * QCW)
                emv = smalls.tile([DK + 1, QCW], f32, tag="emv",
                                  name=f"emv{qc}_{j}")
                nc.vector.tensor_copy(emv, av_ps)
                rsb = smalls.tile([DK + 1, QCW], f32r, tag="rsb",
                                  name=f"rsb{qc}_{j}")
                nc.vector.reciprocal(rsb[DK:DK + 1, :], emv[DK:DK + 1, :])
                rb_ps = miscps.tile([P, QCW], f32, tag="misc", name=f"rb{qc}_{j}")
                nc.tensor.matmul(
                    rb_ps,
                    lhsT=ones_col[DK:DK + 1, :],
                    rhs=rsb[DK:DK + 1, :],
                    start=True,
                    stop=True,
                )
                rbc = smalls.tile([P, QCW], f32, tag="rbc", name=f"rbc{qc}_{j}")
                nc.vector.tensor_copy(rbc, rb_ps)
                nc.vector.tensor_mul(
                    attT_sb[:, j, qsl], emv[0:DK, :], rbc[0:DK, :]
                )
                for t in range(NT):
                    eh, tl = em_at(em, t)
                    if t % 2 == 1:
                        nc.gpsimd.tensor_mul(eh[:, tl, :], eh[:, tl, :], rbc)
                    else:
                        nc.vector.tensor_mul(eh[:, tl, :], eh[:, tl, :], rbc)
                    if t % 4 == 3:
                        tg = t // 4
                        nc.sync.dma_start(
                            attn_t[
                                j, tg * 4 * P:(tg + 1) * 4 * P, qsl
                            ].rearrange("(t p) q -> p t q", p=P),
                            em[tg].bitcast(f32),
                        )

            def emit_fc(qc):
                ot = fcout.tile([P, 4, D], f32, tag="fo", name=f"fo{qc}")
                for i, st in enumerate(range(qc * 4, (qc + 1) * 4)):
                    psf = miscps.tile([P, D], f32, tag="misc", name=f"fc{st}")
                    for j in range(HPC):
                        nc.tensor.matmul(
                            psf,
                            lhsT=attT_sb[:, j, st * P:(st + 1) * P],
                            rhs=wfc_sb[:, j, :],
                            start=(j == 0),
                            stop=(j == HPC - 1),
                        )
                    nc.vector.tensor_copy(ot[:, i, :], psf)
                nc.sync.dma_start(
                    fc_partial[qc * 4 * P:(qc + 1) * 4 * P, :].rearrange(
                        "(t p) d -> p t d", p=P
                    ),
                    ot,
                )

            # --- phase 1: KR partials on this core's n-quarter (full-array
            # --- M=128 matmuls over ALL 8 heads of its batch), exchanged via
            # --- one ReduceScatter; foreign quarter slots zeroed by the
            # --- host-provided one-hot qsel so the program stays SPMD ------
            from contextlib import ExitStack as _ES
            ph1 = _ES()
            rpool = ph1.enter_context(tc.tile_pool(name="rstream", bufs=2))
            krps = ph1.enter_context(
                tc.tile_pool(name="krps", bufs=1, space="PSUM")
            )
            krsb = ph1.enter_context(tc.tile_pool(name="krsb", bufs=1))
            rs_send = dpool_fc.tile([GROUP, HD, GROUP, QCW], f32)
            rs_recv = dpool_fc.tile([HD, GROUP, QCW], f32)
            kr_ps = [
                krps.tile([P, QCW], f32, tag=f"krp{h}", name=f"krp{h}")
                for h in range(2)
            ]
            for ktg in range(NT // 8):
                rt = rpool.tile([P, 8, QCW], bf16, tag="rt")
                nc.sync.dma_start(
                    rt,
                    rel_b[ktg * 8 * P:(ktg + 1) * 8 * P, :].rearrange(
                        "(t p) q -> p t q", p=P
                    ),
                )
                for i in range(8):
                    kt = ktg * 8 + i
                    for h in range(2):
                        nc.tensor.matmul(
                            kr_ps[h],
                            lhsT=K_sb[:, kt, h * P:(h + 1) * P],
                            rhs=rt[:, i, :],
                            start=(kt == 0),
                            stop=(kt == NT - 1),
                        )
            for h in range(2):
                ksb = krsb.tile([P, QCW], f32, tag="ks", name=f"ksb{h}")
                nc.vector.tensor_copy(ksb, kr_ps[h])
                st = krsb.tile([P, GROUP, QCW], f32, tag="st", name=f"st{h}")
                for g in range(GROUP):
                    nc.vector.tensor_scalar_mul(
                        st[:, g, :], ksb, qsel_sb[:, g:g + 1]
                    )
                nc.sync.dma_start(
                    rs_send[2 * h:2 * h + 2].rearrange("m h q w -> (m h) q w"),
                    st,
                )
            if use_collective:
                nc.gpsimd.collective_compute(
                    "ReduceScatter",
                    OP.add,
                    replica_groups=[[0, 1, 2, 3], [4, 5, 6, 7]],
                    ins=[rs_send.opt()],
                    outs=[rs_recv.opt()],
                )
            else:
                nc.sync.dma_start(rs_recv[:], rs_send[0])
            for s in range(NQC):
                nc.sync.dma_start(KR_nc[s], rs_recv[:, s, :].bitcast(f32r))
            ph1.close()

            # --- phase 2: uniform q-chunk pipeline ---
            ctx2 = _ES()
            scps = ctx2.enter_context(
                tc.tile_pool(name="scps", bufs=2, space="PSUM")
            )
            avps = ctx2.enter_context(
                tc.tile_pool(name="avps", bufs=2, space="PSUM")
            )
            miscps = ctx2.enter_context(
                tc.tile_pool(name="miscps", bufs=2, space="PSUM")
            )
            for qc in range(NQC):
                mt = load_mask(qc)
                for j in range(HPC):
                    em = [empool.tile([P, 4, QCW], f32r, tag="em",
                                      name=f"em{qc}_{j}_{h}") for h in range(4)]
                    av_ps = avps.tile([DK + 1, QCW], f32, tag="av",
                                      name=f"av{qc}_{j}")
                    for t in range(NT + SKEW):
                        if t < NT and t % 2 == 0:
                            pair_head(qc, j, em, mt, t)
                        if t >= SKEW:
                            tile_attv(j, em, av_ps, t - SKEW)
                    iter_tail(qc, j, em, av_ps)
                emit_fc(qc)
            ctx2.close()

        # ---------------- phase 3: ReduceScatter + layernorm ----------------
        with (
            tc.tile_pool(name="lnp", bufs=2) as lnp,
            tc.tile_pool(name="lnc", bufs=1) as lnc,
            tc.tile_pool(name="dram", bufs=1, space="DRAM") as dpool,
        ):
            rs_out = dpool.tile([LNR, D], f32)
            if use_collective:
                nc.gpsimd.collective_compute(
                    "ReduceScatter",
                    OP.add,
                    replica_groups=[[0, 1, 2, 3], [4, 5, 6, 7]],
                    ins=[fc_partial.opt()],
                    outs=[rs_out.opt()],
                )
            else:
                # timing-sim variant: stand in for the ReduceScatter with a
                # same-size DRAM->DRAM copy of this core's slice
                nc.sync.dma_start(rs_out[:], fc_partial[0:LNR, :])
            # layernorm over this core's 512-row slice
            rs_sb = lnc.tile([P, 4, D], f32)
            qr_sb = lnc.tile([P, 4, D], f32)
            nc.sync.dma_start(rs_sb, rs_out[:].rearrange("(t p) d -> p t d", p=P))
            nc.sync.dma_start(qr_sb, q_rows.rearrange("(t p) d -> p t d", p=P))
            bfc_bc = lnc.tile([P, D], f32)
            gamma_bc = lnc.tile([P, D], f32)
            beta_bc = lnc.tile([P, D], f32)
            for dst, vec in ((bfc_bc, bfc), (gamma_bc, gamma), (beta_bc, beta)):
                src = bass.AP(
                    tensor=vec.tensor, offset=vec.offset, ap=[[0, P]] + vec.ap
                )
                nc.gpsimd.dma_start(out=dst, in_=src)
            eps_t = lnc.tile([P, 1], f32)
            nc.vector.memset(eps_t, EPS)
            for t in range(4):
                x = lnp.tile([P, D], f32, tag="x")
                nc.vector.tensor_add(x, rs_sb[:, t, :], qr_sb[:, t, :])
                nc.vector.tensor_add(x, x, bfc_bc)
                stats = lnp.tile([P, 6], f32, tag="st")
                nc.vector.bn_stats(stats, x)
                mv = lnp.tile([P, 2], f32, tag="mv")
                nc.vector.bn_aggr(mv, stats)
                std = lnp.tile([P, 1], f32, tag="sd")
                nc.scalar.activation(std, mv[:, 1:2], FN.Sqrt, bias=eps_t)
                rstd = lnp.tile([P, 1], f32, tag="rs")
                nc.vector.reciprocal(rstd, std)
                xn = lnp.tile([P, D], f32, tag="xn")
                nc.vector.tensor_scalar(
                    xn, x, scalar1=mv[:, 0:1], scalar2=rstd,
                    op0=OP.subtract, op1=OP.mult,
                )
                nc.vector.tensor_mul(xn, xn, gamma_bc)
                nc.vector.tensor_add(xn, xn, beta_bc)
                nc.sync.dma_start(out_ln[t * P:(t + 1) * P, :], xn)


def _build(use_collective=True, n_iters=1):
    global _CACHED_NC
    if use_collective and n_iters == 1 and _CACHED_NC is not None:
        return _CACHED_NC
    nc = bacc.Bacc(
        "TRN2",
        target_bir_lowering=False,
        debug=False,
        enable_asserts=True,
        num_devices=8,
    )
    io = {}
    io["q_b"] = nc.dram_tensor("q_b", [D, S], f32r, kind="ExternalInput").ap()
    io["k_b"] = nc.dram_tensor("k_b", [D, S], f32r, kind="ExternalInput").ap()
    io["v_b"] = nc.dram_tensor("v_b", [D, S], f32r, kind="ExternalInput").ap()
    io["rel_b"] = nc.dram_tensor("rel_b", [S, QCW], bf16, kind="ExternalInput").ap()
    io["maskneg_t"] = nc.dram_tensor(
        "maskneg_t", [S, S], f8e5, kind="ExternalInput"
    ).ap()
    io["q_rows"] = nc.dram_tensor("q_rows", [LNR, D], f32, kind="ExternalInput").ap()
    io["wq"] = nc.dram_tensor("wq", [D, HD], f32r, kind="ExternalInput").ap()
    io["wk"] = nc.dram_tensor("wk", [D, D], f32r, kind="ExternalInput").ap()
    io["wv"] = nc.dram_tensor("wv", [D, HD], f32r, kind="ExternalInput").ap()
    io["wfc"] = nc.dram_tensor("wfc", [HD, D], f32r, kind="ExternalInput").ap()
    io["bfc"] = nc.dram_tensor("bfc", [D], f32, kind="ExternalInput").ap()
    io["qsel"] = nc.dram_tensor("qsel", [4], f32, kind="ExternalInput").ap()
    io["gamma"] = nc.dram_tensor("gamma", [D], f32, kind="ExternalInput").ap()
    io["beta"] = nc.dram_tensor("beta", [D], f32, kind="ExternalInput").ap()
    io["attn_t"] = nc.dram_tensor(
        "attn_t", [HPC, S, S], f32, kind="ExternalOutput"
    ).ap()
    io["out_ln"] = nc.dram_tensor("out_ln", [LNR, D], f32, kind="ExternalOutput").ap()

    with tile.TileContext(nc) as tc:
        for _ in range(n_iters):
            _emit(tc, io, use_collective=use_collective)
    nc.compile()
    nc.m = get_hw_module(nc.m)
    if use_collective and n_iters == 1:
        _CACHED_NC = nc
    return nc


def kernel(query, key_, value, mask, relation, Wq, Wk, Wv, Wfc, bfc, gamma, beta,
           _trace=False):
    query = np.asarray(query, np.float32)
    key_ = np.asarray(key_, np.float32)
    value = np.asarray(value, np.float32)
    mask = np.asarray(mask)
    relation = np.asarray(relation, np.float32)
    Wq = np.asarray(Wq, np.float32)
    Wk = np.asarray(Wk, np.float32)
    Wv = np.asarray(Wv, np.float32)
    Wfc = np.asarray(Wfc, np.float32)
    bfc = np.asarray(bfc, np.float32)
    gamma = np.asarray(gamma, np.float32)
    beta = np.asarray(beta, np.float32)

    nc = _build()

    maskneg = {}
    qkvT = {}
    relb16 = {}
    for b in range(B):
        maskneg[b] = np.ascontiguousarray(
            mask[b].T.astype(np.float32) * np.float32(-1024.0)
        ).astype(ml_dtypes.float8_e5m2)
        relb16[b] = np.ascontiguousarray(relation[b]).astype(ml_dtypes.bfloat16)
        qkvT[b] = (
            np.ascontiguousarray(query[b].T),
            np.ascontiguousarray(key_[b].T),
            np.ascontiguousarray(value[b].T),
        )

    in_maps = []
    for c in range(8):
        b, hp = c // 4, c % 4
        in_maps.append({
            "q_b": qkvT[b][0],
            "k_b": qkvT[b][1],
            "v_b": qkvT[b][2],
            "rel_b": np.ascontiguousarray(relb16[b][:, QCW * hp:QCW * (hp + 1)]),
            "maskneg_t": maskneg[b],
            "q_rows": np.ascontiguousarray(query[b, LNR * hp:LNR * (hp + 1)]),
            "wq": np.ascontiguousarray(Wq[:, HD * hp:HD * (hp + 1)]),
            "wk": Wk,
            "wv": np.ascontiguousarray(Wv[:, HD * hp:HD * (hp + 1)]),
            "wfc": np.ascontiguousarray(Wfc[HD * hp:HD * (hp + 1), :]),
            "bfc": bfc,
            "qsel": np.eye(4, dtype=np.float32)[hp],
            "gamma": gamma,
            "beta": beta,
        })

    res = bass_utils.run_bass_kernel_spmd(
        nc, in_maps, core_ids=list(range(8)), trace=_trace
    )

    attn_full = np.empty((B, H, S, S), np.float32)
    out_full = np.empty((B, S, D), np.float32)
    for c in range(8):
        b, hp = c // 4, c % 4
        r = res.results[c]
        for j in range(HPC):
            attn_full[b, HPC * hp + j] = r["attn_t"][j].T
        out_full[b, LNR * hp:LNR * (hp + 1)] = r["out_ln"]
    kernel._last_results = res
    return out_full, attn_full
